# revision 14
# baseline (speedup 1.0000x reference)
"""BitNet FFN kernel for Trainium2, 8 NeuronCores, data-parallel over tokens.

Math (per token row t of x):
  layer1: xn = rmsnorm(x)*g1 ; xq = round(xn*s1)/s1 (int8 grid) ;
          wq1 = tern(w1)/sw1 ; h = xq @ wq1.T ; hp = relu(h)^2
  layer2: same bitlinear on hp with w2, g2.

Device strategy:
  - activation ints (<=127) and ternary weights (+-1/0) are exactly
    representable in bf16 / fp8e4m3, so the matmuls run on the PE array as
    integer-exact bf16 x fp8 with fp32 PSUM accumulation; all scale factors
    are folded into per-token [128,1] scalars applied on the engines.
  - weights are ternarized + packed on host (deterministic prep), activations
    are quantized on device.
  - tokens are processed in groups of 256 (two 128-token partition blocks),
    software-pipelined: quant1 -> matmul1 (w1 streamed) -> relu^2 staged to
    DRAM as scaled fp16 -> per-token scales -> transposed re-read + quantize
    in f-major -> matmul2 (w2 streamed) -> scaled eviction.
  - relu(h)^2 is staged in DRAM as fp16 scaled by 2^-16 (integers <= 2048
    exact; larger values round at 2^-11 relative, which moves the final
    max-rel error only from 0.0130 to 0.0132 -- measured on the real data).
  - DMA ring assignment: sync ring carries ONLY xbar transposes (mode
    switches between transpose and copy serialize the queue); weights, x
    loads and y stores stream on the scalar ring; staging writes go through
    gpsimd (SWDGE).
"""

import sys

for _p in ("/opt/trn_rl_repo", "/root/.axon_site/_ro/trn_rl_repo"):
    if _p not in sys.path:
        sys.path.insert(0, _p)

import numpy as np
import ml_dtypes

import concourse.bass as bass
import concourse.tile as tile
from concourse import bacc, mybir
from concourse.bass_utils import run_bass_kernel_spmd

F32 = mybir.dt.float32
BF16 = mybir.dt.bfloat16
FP16 = mybir.dt.float16
FP8 = mybir.dt.float8e4
NP_FP8 = ml_dtypes.float8_e4m3

N_CORES = 8
D = 2048          # d_model
F = 8192          # d_ff
B, S = 4, 2048
T_TOTAL = B * S
T_CORE = T_TOTAL // N_CORES   # 1024 tokens per core

EPS_NORM = 1e-6
EPS_SCALE = 1e-5
MAGIC = 12582912.0            # 1.5 * 2**23: fp32 round-to-nearest-even trick
HSC = 2.0 ** -8               # relu pre-scale; staged r' = (relu(h)*HSC)^2

STAGE_ENG = "scalar"          # engine for staging writes: gpsimd | scalar | sync
H_DT = FP16                   # dtype for staged relu^2 (FP16 or BF16)
PLAIN_EVICT = True            # fused ttr/accum_out instructions crash on HW

TB = 128                      # tokens per partition block
NTB_G = 2                     # token blocks per group
TG = TB * NTB_G               # 256 tokens per pipelined group
ORNG = 512                    # matmul1 moving free width over d_ff
N_ORNG = F // ORNG            # 16
N_I = D // 128                # 16 contraction chunks, layer1
N_DSL = D // 512              # 4 output d slices, layer2
N_OCH = F // 128              # 64 contraction chunks, layer2
NQ = N_OCH // 4               # w2 quarter tile chunk count


def build_nc(t_core: int, unit_g: bool = True, replicas: int = 0):
    """Build the per-core Bass program for t_core tokens."""
    n_g = t_core // TG
    nc = bacc.Bacc("TRN2")
    stage_eng = getattr(nc, STAGE_ENG)

    x_ext = nc.declare_dram_parameter("x", [t_core, D], F32, isOutput=False)
    # packed ternary weights (see _prep_weights): fp8, +-1/0
    w1_ext = nc.declare_dram_parameter("w1p", [N_ORNG, 128, N_I, ORNG], FP8, isOutput=False)
    w2_ext = nc.declare_dram_parameter("w2p", [N_DSL, 128, N_OCH, 512], FP8, isOutput=False)
    g1_ext = nc.declare_dram_parameter("g1", [D], F32, isOutput=False)
    g2f_ext = nc.declare_dram_parameter("g2f", [F], F32, isOutput=False)
    # [W1S/127, W2S/127] where WkS = clip(mean|wk|, eps) (weight dequant)
    ws_ext = nc.declare_dram_parameter("wsc", [2], F32, isOutput=False)
    # g2 in f-major layout [p, och] for the general (unit_g=False) path
    g2t_ext = nc.declare_dram_parameter("g2t", [128, N_OCH], F32, isOutput=False)
    y_ext = nc.declare_dram_parameter("y", [t_core, D], F32, isOutput=True)

    def bcast(ap, p=128):
        return bass.AP(tensor=ap.tensor, offset=ap.offset, ap=[[0, p]] + list(ap.ap))

    import contextlib
    with tile.TileContext(nc) as tc:
        with contextlib.ExitStack() as _stack:
            def _pool(name, bufs, space="SBUF"):
                return _stack.enter_context(
                    tc.tile_pool(name=name, bufs=bufs, space=space))
            singles = _pool("singles", 1)
            xin_p = _pool("xin", 2)
            xq_p = _pool("xq", 2)
            xqt_p = _pool("xqt", 2)
            x2t_p = _pool("x2t", 2)
            w1_p = _pool("w1", 4)
            w2_p = _pool("w2", 5)
            rl_p = _pool("rl", 3)
            hb_p = _pool("hb", 4)
            tr_p = _pool("tr", 2)
            htj_p = _pool("htj", 4)
            tq_p = _pool("tq", 3)
            taur_p = _pool("taur", 2)
            st_p = _pool("st", 2)
            sc_p = _pool("sc", 3)
            dq2_p = _pool("dq2k", 2 * NTB_G)
            y_p = _pool("yb", 2)
            ps1_p = _pool("ps1", 6, space="PSUM")
            ps2_p = _pool("ps2", 2, space="PSUM")
            dram_p = _pool("dstage", 2, space="DRAM")
            ws_rep = singles.tile([128, 2], F32)
            nc.scalar.dma_start(out=ws_rep[:], in_=bcast(ws_ext[:]))
            eps_n = singles.tile([128, 1], F32)
            nc.vector.memset(eps_n[:], EPS_NORM)
            if not unit_g:
                g1_rep = singles.tile([128, D], F32)
                nc.scalar.dma_start(out=g1_rep[:], in_=bcast(g1_ext[:]))
                g2_rep = singles.tile([128, F], F32)
                nc.scalar.dma_start(out=g2_rep[:], in_=bcast(g2f_ext[:]))
                g2t = singles.tile([128, N_OCH], F32)
                nc.scalar.dma_start(out=g2t[:], in_=g2t_ext[:])

            def quant1(g):
                """Load x for group g, rmsnorm+int8-quantize, DMA-transpose
                to x_qT [128(i), N_I, TG] bf16. Returns (xqT, dq1sq, dq14)."""
                xq_stage = dram_p.tile([TG, D], BF16, tag="xqs", name="xqs")
                xqT = xqt_p.tile([128, N_I, TG], BF16, tag="xqT")
                dq1sq = sc_p.tile([128, NTB_G], F32, tag="dq1sq")
                dq14 = sc_p.tile([128, NTB_G], F32, tag="dq14")
                for tb in range(NTB_G):
                    t0 = g * TG + tb * TB
                    xb = xin_p.tile([128, D], F32, tag="xb")
                    nc.scalar.dma_start(out=xb[:], in_=x_ext[t0:t0 + TB, :])
                    stats = st_p.tile([128, D // 512, 6], F32, tag="st1")
                    xbv = xb.rearrange("p (c f) -> p c f", f=512)
                    for c in range(D // 512):
                        nc.vector.bn_stats(out=stats[:, c, :], in_=xbv[:, c, :])
                    mv = sc_p.tile([128, 2], F32, tag="mv1")
                    nc.vector.bn_aggr(out=mv[:], in_=stats[:])
                    e1 = sc_p.tile([128, 1], F32, tag="e1")
                    nc.vector.tensor_mul(e1[:], mv[:, 0:1], mv[:, 0:1])
                    nc.vector.tensor_add(e1[:], e1[:], mv[:, 1:2])
                    rms = sc_p.tile([128, 1], F32, tag="rms1")
                    nc.scalar.activation(out=rms[:], in_=e1[:],
                                         func=mybir.ActivationFunctionType.Sqrt,
                                         bias=eps_n[:], scale=1.0)
                    rinv = sc_p.tile([128, 1], F32, tag="rinv1")
                    nc.vector.reciprocal(out=rinv[:], in_=rms[:])
                    if not unit_g:
                        # xg = x * g1 (in place; raw x no longer needed)
                        nc.vector.tensor_mul(xb[:], xb[:], g1_rep[:])
                    am = sc_p.tile([128, 1], F32, tag="am1")
                    nc.vector.tensor_reduce(out=am[:], in_=xb[:],
                                            axis=mybir.AxisListType.X,
                                            op=mybir.AluOpType.max,
                                            apply_absolute_value=True)
                    nc.vector.tensor_mul(am[:], am[:], rinv[:])
                    c1 = sc_p.tile([128, 1], F32, tag="c1")
                    nc.vector.tensor_scalar_max(c1[:], am[:], EPS_SCALE)
                    ic1 = sc_p.tile([128, 1], F32, tag="ic1")
                    nc.vector.reciprocal(out=ic1[:], in_=c1[:])
                    q1 = sc_p.tile([128, 1], F32, tag="q1")
                    nc.vector.tensor_mul(q1[:], rinv[:], ic1[:])
                    nc.vector.tensor_scalar_mul(q1[:], q1[:], 127.0)
                    dq1 = sc_p.tile([128, 1], F32, tag="dq1")
                    nc.vector.tensor_mul(dq1[:], c1[:], ws_rep[:, 0:1])
                    nc.vector.tensor_mul(dq1sq[:, tb:tb + 1], dq1[:], dq1[:])
                    nc.vector.tensor_mul(dq14[:, tb:tb + 1],
                                         dq1sq[:, tb:tb + 1], dq1sq[:, tb:tb + 1])
                    # round(xg * q1) via magic-number RNE, cast to bf16
                    nc.vector.tensor_scalar(xb[:], xb[:], q1[:], MAGIC,
                                            op0=mybir.AluOpType.mult,
                                            op1=mybir.AluOpType.add)
                    xqb = xq_p.tile([128, D], BF16, tag="xqb")
                    nc.scalar.activation(out=xqb[:], in_=xb[:],
                                         func=mybir.ActivationFunctionType.Copy,
                                         bias=-MAGIC, scale=1.0)
                    stage_eng.dma_start(out=xq_stage[tb * TB:(tb + 1) * TB, :],
                                        in_=xqb[:])
                for i in range(N_I):
                    nc.sync.dma_start(out=xqT[:, i, :],
                                      in_=xq_stage[:, i * 128:(i + 1) * 128],
                                      transpose=True)
                return xqT, dq1sq, dq14

            def matmul1(g, xqT, h_stage, ss, am2):
                """h_int = x_int @ w1tern^T for group g. Evict r' =
                (relu(h)*2^-8)^2 to DRAM fp16 plus per-chunk sum(r'^2) and
                max(r')."""
                for orng in range(N_ORNG):
                    w1t = w1_p.tile([128, N_I, ORNG], FP8, tag="w1t")
                    nc.scalar.dma_start(out=w1t[:], in_=w1_ext[orng])
                    for tb in range(NTB_G):
                        pss = ps1_p.tile([128, ORNG], F32, tag="ps1")
                        for i in range(N_I):
                            nc.tensor.matmul(pss[:],
                                             lhsT=xqT[:, i, tb * TB:(tb + 1) * TB],
                                             rhs=w1t[:, i, :],
                                             start=(i == 0), stop=(i == N_I - 1))
                        rl = rl_p.tile([128, ORNG], F32, tag="rl")
                        nc.scalar.activation(out=rl[:], in_=pss[:],
                                             func=mybir.ActivationFunctionType.Relu,
                                             scale=HSC)
                        # hb = rl*rl (fp16) and chunk max(r') in one DVE op
                        hb = hb_p.tile([128, ORNG], H_DT, tag="hb")
                        if PLAIN_EVICT:
                            nc.vector.tensor_mul(hb[:], rl[:], rl[:])
                            nc.vector.tensor_reduce(
                                out=am2[:, tb, orng:orng + 1], in_=hb[:],
                                axis=mybir.AxisListType.X,
                                op=mybir.AluOpType.max)
                            trash = tr_p.tile([128, ORNG], F32, tag="trash")
                            nc.scalar.activation(
                                out=trash[:], in_=hb[:],
                                func=mybir.ActivationFunctionType.Square)
                            nc.vector.tensor_reduce(
                                out=ss[:, tb, orng:orng + 1], in_=trash[:],
                                axis=mybir.AxisListType.X,
                                op=mybir.AluOpType.add)
                        else:
                            nc.vector.tensor_tensor_reduce(
                                out=hb[:], in0=rl[:], in1=rl[:], scale=1.0,
                                scalar=0.0, op0=mybir.AluOpType.mult,
                                op1=mybir.AluOpType.max,
                                accum_out=am2[:, tb, orng:orng + 1])
                            # sum(r'^2) per chunk on the scalar engine
                            trash = tr_p.tile([128, ORNG], F32, tag="trash")
                            nc.scalar.activation(
                                out=trash[:], in_=hb[:],
                                func=mybir.ActivationFunctionType.Square,
                                accum_out=ss[:, tb, orng:orng + 1])
                        if not unit_g:
                            # am2 must be max|r' * g2|; redo it with g2 applied
                            osl = slice(orng * ORNG, (orng + 1) * ORNG)
                            hbg = tr_p.tile([128, ORNG], F32, tag="hbg")
                            nc.vector.tensor_mul(hbg[:], hb[:], g2_rep[:, osl])
                            nc.vector.tensor_reduce(
                                out=am2[:, tb, orng:orng + 1], in_=hbg[:],
                                axis=mybir.AxisListType.X,
                                op=mybir.AluOpType.max,
                                apply_absolute_value=True)
                        stage_eng.dma_start(
                            out=h_stage[tb * TB:(tb + 1) * TB,
                                        orng * ORNG:(orng + 1) * ORNG],
                            in_=hb[:])

            def quant2_scalars(g, ss, am2, dq1sq, dq14, tau_stage):
                """Per-token second-layer quant scales from the staged-r'
                statistics. Writes tau' to DRAM for broadcast; returns dq2
                tiles."""
                dq2_tiles = []
                for tb in range(NTB_G):
                    s2s = sc_p.tile([128, 1], F32, tag="s2s")
                    nc.vector.tensor_reduce(out=s2s[:], in_=ss[:, tb, :],
                                            axis=mybir.AxisListType.X,
                                            op=mybir.AluOpType.add)
                    e2 = sc_p.tile([128, 1], F32, tag="e2")
                    nc.vector.tensor_mul(e2[:], s2s[:], dq14[:, tb:tb + 1])
                    # rms2 = sqrt(E[r_ref^2] + eps); r' = r * 2^-16 so the
                    # 2^32 rescale and the 1/F mean fold into the ACT scale
                    rms = sc_p.tile([128, 1], F32, tag="rms2")
                    nc.scalar.activation(out=rms[:], in_=e2[:],
                                         func=mybir.ActivationFunctionType.Sqrt,
                                         bias=eps_n[:], scale=2.0 ** 32 / F)
                    rinv = sc_p.tile([128, 1], F32, tag="rinv2")
                    nc.vector.reciprocal(out=rinv[:], in_=rms[:])
                    dr = sc_p.tile([128, 1], F32, tag="dr")   # dq1^2 * rinv2
                    nc.vector.tensor_mul(dr[:], dq1sq[:, tb:tb + 1], rinv[:])
                    amr = sc_p.tile([128, 1], F32, tag="amr")
                    nc.vector.tensor_reduce(out=amr[:], in_=am2[:, tb, :],
                                            axis=mybir.AxisListType.X,
                                            op=mybir.AluOpType.max)
                    nc.vector.tensor_mul(amr[:], amr[:], dr[:])
                    # c2 = max(am_ref * rinv2, eps); amr holds it * 2^-16
                    c2 = sc_p.tile([128, 1], F32, tag="c2")
                    nc.vector.tensor_scalar(c2[:], amr[:], 65536.0, EPS_SCALE,
                                            op0=mybir.AluOpType.mult,
                                            op1=mybir.AluOpType.max)
                    ic2 = sc_p.tile([128, 1], F32, tag="ic2")
                    nc.vector.reciprocal(out=ic2[:], in_=c2[:])
                    # tau' = 127 * 2^16 * dq1^2 * rinv2 / c2  (applied to r')
                    tau = sc_p.tile([128, 1], F32, tag="tau")
                    nc.vector.tensor_mul(tau[:], dr[:], ic2[:])
                    nc.vector.tensor_scalar_mul(tau[:], tau[:], 127.0 * 65536.0)
                    stage_eng.dma_start(
                        out=tau_stage[tb * TB:(tb + 1) * TB, :], in_=tau[:])
                    dq2 = dq2_p.tile([128, 1], F32, tag="dq2k", name=f"dq2_{tb}")
                    nc.vector.tensor_mul(dq2[:], c2[:], ws_rep[:, 1:2])
                    dq2_tiles.append(dq2)
                return dq2_tiles

            def fquant(g, h_stage, tau_stage):
                """Transposed re-read of r' + f-major quantization into
                x2T [128(f), N_OCH, TG] bf16 (int8-grid values)."""
                tau_rep = taur_p.tile([128, TG], F32, tag="taur")
                ts_ap = tau_stage[:]
                nc.scalar.dma_start(out=tau_rep[:], in_=bass.AP(
                    tensor=ts_ap.tensor, offset=ts_ap.offset,
                    ap=[[0, 128], [1, TG]]))
                x2T = x2t_p.tile([128, N_OCH, TG], BF16, tag="x2T")
                for j in range(N_OCH):
                    htj = htj_p.tile([128, TG], H_DT, tag="htj")
                    nc.sync.dma_start(out=htj[:],
                                      in_=h_stage[:, j * 128:(j + 1) * 128],
                                      transpose=True)
                    tmpq = tq_p.tile([128, TG], F32, tag="tmpq")
                    nc.vector.tensor_mul(tmpq[:], htj[:], tau_rep[:])
                    if unit_g:
                        nc.vector.tensor_scalar_add(tmpq[:], tmpq[:], MAGIC)
                    else:
                        nc.vector.tensor_scalar(tmpq[:], tmpq[:],
                                                g2t[:, j:j + 1], MAGIC,
                                                op0=mybir.AluOpType.mult,
                                                op1=mybir.AluOpType.add)
                    nc.scalar.activation(out=x2T[:, j, :], in_=tmpq[:],
                                         func=mybir.ActivationFunctionType.Copy,
                                         bias=-MAGIC, scale=1.0)
                return x2T

            def w2_load(dsl, g):
                tiles = []
                for h in range(4):
                    w2h = w2_p.tile([128, NQ, 512], FP8, tag="w2t",
                                    name=f"w2t_{g}_{dsl}_{h}")
                    nc.scalar.dma_start(
                        out=w2h[:], in_=w2_ext[dsl, :, h * NQ:(h + 1) * NQ, :])
                    tiles.append(w2h)
                return tiles

            def matmul2(g, x2T, dq2_tiles, w2_first):
                w2_cur = w2_first
                for dsl in range(N_DSL):
                    w2_next = w2_load(dsl + 1, g) if dsl + 1 < N_DSL else None
                    for tb in range(NTB_G):
                        pys = ps2_p.tile([128, 512], F32, tag="ps2")
                        for och in range(N_OCH):
                            h, j = divmod(och, NQ)
                            nc.tensor.matmul(pys[:],
                                             lhsT=x2T[:, och, tb * TB:(tb + 1) * TB],
                                             rhs=w2_cur[h][:, j, :],
                                             start=(och == 0), stop=(och == N_OCH - 1))
                        yt = y_p.tile([128, 512], F32, tag="yt")
                        nc.vector.tensor_scalar_mul(yt[:], pys[:], dq2_tiles[tb][:])
                        t0 = g * TG + tb * TB
                        nc.scalar.dma_start(
                            out=y_ext[t0:t0 + TB, dsl * 512:(dsl + 1) * 512],
                            in_=yt[:])
                    w2_cur = w2_next

            def pipeline():
                q1_out = [None] * n_g
                mm1_out = [None] * n_g

                def p1(g):
                    xqT, dq1sq, dq14 = q1_out[g]
                    h_stage = dram_p.tile([TG, F], H_DT, tag="hs", name=f"hs_{g}")
                    ss = st_p.tile([128, NTB_G, N_ORNG], F32, tag="ss")
                    am2 = st_p.tile([128, NTB_G, N_ORNG], F32, tag="am2")
                    matmul1(g, xqT, h_stage, ss, am2)
                    mm1_out[g] = (h_stage, ss, am2)

                q1_out[0] = quant1(0)
                if n_g > 1:
                    q1_out[1] = quant1(1)
                p1(0)
                for g in range(n_g):
                    xqT, dq1sq, dq14 = q1_out[g]
                    h_stage, ss, am2 = mm1_out[g]
                    tau_stage = dram_p.tile([TG, 1], F32, tag="taus",
                                            name=f"taus_{g}")
                    dq2s = quant2_scalars(g, ss, am2, dq1sq, dq14, tau_stage)
                    w2_first = w2_load(0, g)
                    x2T = fquant(g, h_stage, tau_stage)
                    if g + 2 < n_g:
                        q1_out[g + 2] = quant1(g + 2)
                    if g + 1 < n_g:
                        p1(g + 1)
                    matmul2(g, x2T, dq2s, w2_first)

            import contextlib
            loop_ctx = tc.For_i(0, replicas, 1) if replicas > 0 else contextlib.nullcontext()
            with loop_ctx:
                pipeline()

    nc.finalize()
    return nc


_NC_CACHE: dict = {}


def _get_nc(t_core: int, unit_g: bool = True):
    key = (t_core, unit_g)
    if key not in _NC_CACHE:
        _NC_CACHE[key] = build_nc(t_core, unit_g)
    return _NC_CACHE[key]


def _prep_weights(w1: np.ndarray, w2: np.ndarray):
    """Host ternarization + tiling. Returns (w1p, w2p, wsc)."""
    def tern(w):
        ws = max(float(np.mean(np.abs(w.astype(np.float64)))), EPS_SCALE)
        t = np.clip(np.round(w.astype(np.float64) / ws), -1, 1).astype(np.float32)
        return t, ws

    t1, ws1 = tern(w1)          # [F, D]
    t2, ws2 = tern(w2)          # [D, F]
    # matmul1 moving tiles: [o_rng][p=i%128][i_chunk][o_in 512] of w1T[i,o]
    w1T = np.ascontiguousarray(t1.T)                    # [D, F]
    w1p = (w1T.reshape(N_I, 128, N_ORNG, ORNG)          # (i_c, p, o_rng, o_in)
              .transpose(2, 1, 0, 3).astype(NP_FP8))    # [o_rng, p, i_c, o_in]
    w1p = np.ascontiguousarray(w1p)
    # matmul2 moving tiles: [d_slice][o_chunk][p=o%128][d_in 512] of w2T[o,d]
    w2p = (t2.reshape(N_DSL, 512, N_OCH, 128)           # (dsl, d_in, o_c, p)
             .transpose(0, 3, 2, 1).astype(NP_FP8))     # [dsl, p, o_c, d_in]
    w2p = np.ascontiguousarray(w2p)
    wsc = np.array([ws1 / 127.0, ws2 / 127.0], dtype=np.float32)
    return w1p, w2p, wsc


def kernel(x: np.ndarray, w1: np.ndarray, g1: np.ndarray,
           w2: np.ndarray, g2: np.ndarray) -> np.ndarray:
    x = np.asarray(x, dtype=np.float32)
    b, s, d = x.shape
    assert (b, s, d) == (B, S, D), (b, s, d)
    w1p, w2p, wsc = _prep_weights(np.asarray(w1, np.float32),
                                  np.asarray(w2, np.float32))
    g1 = np.asarray(g1, np.float32)
    g2f = np.asarray(g2, np.float32)
    unit_g = bool(np.all(g1 == 1.0) and np.all(g2f == 1.0))

    g2t_host = np.ascontiguousarray(g2f.reshape(N_OCH, 128).T)
    xt = x.reshape(T_TOTAL, D)
    nc = _get_nc(T_CORE, unit_g)
    in_maps = []
    for c in range(N_CORES):
        in_maps.append({
            "x": np.ascontiguousarray(xt[c * T_CORE:(c + 1) * T_CORE]),
            "w1p": w1p, "w2p": w2p, "g1": g1, "g2f": g2f, "wsc": wsc,
            "g2t": g2t_host,
        })
    res = run_bass_kernel_spmd(nc, in_maps, list(range(N_CORES)))
    outs = [np.asarray(res.results[c]["y"], np.float32) for c in range(N_CORES)]
    y = np.concatenate(outs, axis=0).reshape(B, S, D)
    return y


# revision 16
# speedup vs baseline: 5.2282x; 5.2282x over previous
"""BitNet FFN kernel for Trainium2, 8 NeuronCores, data-parallel over tokens.

Math (per token row t of x):
  layer1: xn = rmsnorm(x)*g1 ; xq = round(xn*s1)/s1 (int8 grid) ;
          wq1 = tern(w1)/sw1 ; h = xq @ wq1.T ; hp = relu(h)^2
  layer2: same bitlinear on hp with w2, g2.

Device strategy:
  - activation ints (<=127) and ternary weights (+-1/0) are exactly
    representable in bf16 / fp8e4m3, so the matmuls run on the PE array as
    integer-exact bf16 x fp8 with fp32 PSUM accumulation; all scale factors
    are folded into per-token [128,1] scalars applied on the engines.
  - weights are ternarized + packed on host (deterministic prep), activations
    are quantized on device.
  - tokens are processed in groups of 256 (two 128-token partition blocks),
    software-pipelined: quant1 -> matmul1 (w1 streamed) -> relu^2/stats ->
    quant2 -> DMA-transpose -> matmul2 (w2 streamed) -> scaled eviction.
"""

import sys

for _p in ("/opt/trn_rl_repo", "/root/.axon_site/_ro/trn_rl_repo"):
    if _p not in sys.path:
        sys.path.insert(0, _p)

import numpy as np
import ml_dtypes

import concourse.bass as bass
import concourse.tile as tile
from concourse import bacc, mybir
from concourse.bass_utils import run_bass_kernel_spmd

F32 = mybir.dt.float32
BF16 = mybir.dt.bfloat16
FP8 = mybir.dt.float8e4
NP_FP8 = ml_dtypes.float8_e4m3
NP_BF16 = ml_dtypes.bfloat16

N_CORES = 8
D = 2048          # d_model
F = 8192          # d_ff
B, S = 4, 2048
T_TOTAL = B * S
T_CORE = T_TOTAL // N_CORES   # 1024 tokens per core

EPS_NORM = 1e-6
EPS_SCALE = 1e-5
MAGIC = 12582912.0            # 1.5 * 2**23: fp32 round-to-nearest-even trick

TB = 128                      # tokens per partition block
NTB_G = 2                     # token blocks per group
TG = TB * NTB_G               # 256 tokens per pipelined group
ORNG = 512                    # matmul1 moving free width over d_ff
N_ORNG = F // ORNG            # 16
N_I = D // 128                # 16 contraction chunks, layer1
N_DSL = D // 512              # 4 output d slices, layer2
N_OCH = F // 128              # 64 contraction chunks, layer2


def build_nc(t_core: int, unit_g: bool = True, replicas: int = 0, fake_transpose: bool = False, pe_transpose: bool = False):
    """Build the per-core Bass program for t_core tokens.

    unit_g=True specializes for g1 == g2 == 1 (skips the gain multiplies);
    the general path multiplies by broadcast gain rows.
    replicas>0 wraps the whole pipeline in an on-device For_i loop executing
    it `replicas` times (for HW timing via build-pair differencing).
    """
    n_g = t_core // TG
    nc = bacc.Bacc("TRN2")

    x_ext = nc.declare_dram_parameter("x", [t_core, D], F32, isOutput=False)
    # packed ternary weights (see _prep_weights): fp8, +-1/0
    w1_ext = nc.declare_dram_parameter("w1p", [N_ORNG, 128, N_I, ORNG], FP8, isOutput=False)
    w2_ext = nc.declare_dram_parameter("w2p", [N_DSL, 128, N_OCH, 512], FP8, isOutput=False)
    g1_ext = nc.declare_dram_parameter("g1", [D], F32, isOutput=False)
    g2f_ext = nc.declare_dram_parameter("g2f", [F], F32, isOutput=False)
    # [W1S/127, W2S/127] where WkS = clip(mean|wk|, eps) (weight dequant)
    ws_ext = nc.declare_dram_parameter("wsc", [2], F32, isOutput=False)
    y_ext = nc.declare_dram_parameter("y", [t_core, D], F32, isOutput=True)

    def bcast(ap, p=128):
        return bass.AP(tensor=ap.tensor, offset=ap.offset, ap=[[0, p]] + list(ap.ap))

    nbuf = 2 if unit_g else 1

    with tile.TileContext(nc) as tc:
        with (
            tc.tile_pool(name="singles", bufs=1) as singles,
            tc.tile_pool(name="xin", bufs=nbuf) as xin_p,
            tc.tile_pool(name="xq", bufs=1) as xq_p,
            tc.tile_pool(name="xqt", bufs=nbuf) as xqt_p,
            tc.tile_pool(name="hg", bufs=1) as hg_p,
            tc.tile_pool(name="x2", bufs=1) as x2_p,
            tc.tile_pool(name="x2t", bufs=1) as x2t_p,
            tc.tile_pool(name="w1", bufs=2 if unit_g else 1) as w1_p,
            tc.tile_pool(name="w2", bufs=4 if unit_g else 2) as w2_p,
            tc.tile_pool(name="ev", bufs=3 if unit_g else 2) as ev_p,
            tc.tile_pool(name="st", bufs=2) as st_p,
            tc.tile_pool(name="sc", bufs=2) as sc_p,
            tc.tile_pool(name="dq2k", bufs=2 * NTB_G) as dq2_p,
            tc.tile_pool(name="yb", bufs=2) as y_p,
            tc.tile_pool(name="ps1", bufs=4 if not pe_transpose else 3, space="PSUM") as ps1_p,
            tc.tile_pool(name="ps2", bufs=2, space="PSUM") as ps2_p,
            tc.tile_pool(name="pst", bufs=3, space="PSUM") as pst_p,
            tc.tile_pool(name="dstage", bufs=2, space="DRAM") as dram_p,
        ):
            ws_rep = singles.tile([128, 2], F32)
            nc.sync.dma_start(out=ws_rep[:], in_=bcast(ws_ext[:]))
            eps_n = singles.tile([128, 1], F32)
            nc.vector.memset(eps_n[:], EPS_NORM)
            ident = singles.tile([128, 128], BF16)
            from concourse.masks import make_identity
            make_identity(nc, ident[:])
            if not unit_g:
                g1_rep = singles.tile([128, D], F32)
                nc.sync.dma_start(out=g1_rep[:], in_=bcast(g1_ext[:]))
                g2_rep = singles.tile([128, F], F32)
                nc.sync.dma_start(out=g2_rep[:], in_=bcast(g2f_ext[:]))

            def quant1(g):
                """Load x for group g, rmsnorm+int8-quantize, DMA-transpose
                to x_qT [128(i), N_I, TG] bf16. Returns (xqT, dq1sq, dq14)."""
                xq_stage = None if pe_transpose else dram_p.tile([TG, D], BF16, tag="xqs", name="xqs")
                xqT = xqt_p.tile([128, N_I, TG], BF16, tag="xqT")
                dq1sq = sc_p.tile([128, NTB_G], F32, tag="dq1sq")
                dq14 = sc_p.tile([128, NTB_G], F32, tag="dq14")
                for tb in range(NTB_G):
                    t0 = g * TG + tb * TB
                    xb = xin_p.tile([128, D], F32, tag="xb")
                    nc.sync.dma_start(out=xb[:], in_=x_ext[t0:t0 + TB, :])
                    stats = st_p.tile([128, D // 512, 6], F32, tag="st1")
                    xbv = xb.rearrange("p (c f) -> p c f", f=512)
                    for c in range(D // 512):
                        nc.vector.bn_stats(out=stats[:, c, :], in_=xbv[:, c, :])
                    mv = sc_p.tile([128, 2], F32, tag="mv1")
                    nc.vector.bn_aggr(out=mv[:], in_=stats[:])
                    e1 = sc_p.tile([128, 1], F32, tag="e1")
                    nc.vector.tensor_mul(e1[:], mv[:, 0:1], mv[:, 0:1])
                    nc.vector.tensor_add(e1[:], e1[:], mv[:, 1:2])
                    rms = sc_p.tile([128, 1], F32, tag="rms1")
                    nc.scalar.activation(out=rms[:], in_=e1[:],
                                         func=mybir.ActivationFunctionType.Sqrt,
                                         bias=eps_n[:], scale=1.0)
                    rinv = sc_p.tile([128, 1], F32, tag="rinv1")
                    nc.vector.reciprocal(out=rinv[:], in_=rms[:])
                    if not unit_g:
                        # xg = x * g1 (in place; raw x no longer needed)
                        nc.vector.tensor_mul(xb[:], xb[:], g1_rep[:])
                    am = sc_p.tile([128, 1], F32, tag="am1")
                    nc.vector.tensor_reduce(out=am[:], in_=xb[:],
                                            axis=mybir.AxisListType.X,
                                            op=mybir.AluOpType.max,
                                            apply_absolute_value=True)
                    nc.vector.tensor_mul(am[:], am[:], rinv[:])
                    c1 = sc_p.tile([128, 1], F32, tag="c1")
                    nc.vector.tensor_scalar_max(c1[:], am[:], EPS_SCALE)
                    ic1 = sc_p.tile([128, 1], F32, tag="ic1")
                    nc.vector.reciprocal(out=ic1[:], in_=c1[:])
                    q1 = sc_p.tile([128, 1], F32, tag="q1")
                    nc.vector.tensor_mul(q1[:], rinv[:], ic1[:])
                    nc.vector.tensor_scalar_mul(q1[:], q1[:], 127.0)
                    dq1 = sc_p.tile([128, 1], F32, tag="dq1")
                    nc.vector.tensor_mul(dq1[:], c1[:], ws_rep[:, 0:1])
                    nc.vector.tensor_mul(dq1sq[:, tb:tb + 1], dq1[:], dq1[:])
                    nc.vector.tensor_mul(dq14[:, tb:tb + 1],
                                         dq1sq[:, tb:tb + 1], dq1sq[:, tb:tb + 1])
                    # round(xg * q1) via magic-number RNE, cast to bf16
                    nc.vector.tensor_scalar(xb[:], xb[:], q1[:], MAGIC,
                                            op0=mybir.AluOpType.mult,
                                            op1=mybir.AluOpType.add)
                    xqb = xq_p.tile([128, D], BF16, tag="xqb")
                    nc.scalar.activation(out=xqb[:], in_=xb[:],
                                         func=mybir.ActivationFunctionType.Copy,
                                         bias=-MAGIC, scale=1.0)
                    if pe_transpose:
                        for i in range(N_I):
                            pt = pst_p.tile([128, 128], BF16, tag="pst", name=f"pt1_{i}")
                            nc.tensor.transpose(pt[:], xqb[:, i * 128:(i + 1) * 128],
                                                ident[:])
                            nc.vector.tensor_copy(
                                out=xqT[:, i, tb * TB:(tb + 1) * TB], in_=pt[:])
                    else:
                        nc.sync.dma_start(out=xq_stage[tb * TB:(tb + 1) * TB, :],
                                          in_=xqb[:])
                if not pe_transpose:
                    for i in range(N_I):
                        if fake_transpose:
                            nc.sync.dma_start(out=xqT[:, i, :],
                                              in_=xq_stage[0:128, 0:TG])
                        else:
                            nc.sync.dma_start(out=xqT[:, i, :],
                                              in_=xq_stage[:, i * 128:(i + 1) * 128],
                                              transpose=True)
                return xqT, dq1sq, dq14

            def matmul1(g, xqT, hg, st2, am2):
                """h_int = x_int @ w1tern^T for group g; evict relu^2 (times g2)
                into hg plus per-chunk stats."""
                for orng in range(N_ORNG):
                    w1t = w1_p.tile([128, N_I, ORNG], FP8, tag="w1t")
                    nc.sync.dma_start(out=w1t[:], in_=w1_ext[orng])
                    pss = [ps1_p.tile([128, ORNG], F32, tag="ps1", name=f"ps1_{tb}")
                           for tb in range(NTB_G)]
                    for i in range(N_I):
                        for tb in range(NTB_G):
                            nc.tensor.matmul(pss[tb][:],
                                             lhsT=xqT[:, i, tb * TB:(tb + 1) * TB],
                                             rhs=w1t[:, i, :],
                                             start=(i == 0), stop=(i == N_I - 1))
                    osl = slice(orng * ORNG, (orng + 1) * ORNG)
                    for tb in range(NTB_G):
                        rl = ev_p.tile([128, ORNG], F32, tag="rl")
                        nc.scalar.activation(out=rl[:], in_=pss[tb][:],
                                             func=mybir.ActivationFunctionType.Relu)
                        # hg = relu(h)^2 (times g2 in the general path); the
                        # rms2 stats use r = relu^2 itself, the absmax uses r*g2
                        nc.vector.tensor_mul(hg[:, tb, osl], rl[:], rl[:])
                        nc.vector.bn_stats(out=st2[:, tb, orng, :], in_=hg[:, tb, osl])
                        if not unit_g:
                            nc.vector.tensor_mul(hg[:, tb, osl], hg[:, tb, osl],
                                                 g2_rep[:, osl])
                        nc.vector.tensor_reduce(out=am2[:, tb, orng:orng + 1],
                                                in_=hg[:, tb, osl],
                                                axis=mybir.AxisListType.X,
                                                op=mybir.AluOpType.max,
                                                apply_absolute_value=True)

            def quant2(g, hg, st2, am2, dq1sq, dq14):
                """Second rmsnorm+quant; factors folded into per-token scalars.
                Consumes hg in place. Returns (x2T_or_stage, dq2_tiles)."""
                if pe_transpose:
                    x2_stage = None
                    x2T = x2t_p.tile([128, N_OCH, TG], BF16, tag="x2T", name="x2T")
                else:
                    x2_stage = dram_p.tile([TG, F], BF16, tag="x2s", name="x2s")
                    x2T = None
                dq2_tiles = []
                for tb in range(NTB_G):
                    mv = sc_p.tile([128, 2], F32, tag="mv2")
                    nc.vector.bn_aggr(out=mv[:], in_=st2[:, tb])
                    e2 = sc_p.tile([128, 1], F32, tag="e2")
                    nc.vector.tensor_mul(e2[:], mv[:, 0:1], mv[:, 0:1])
                    nc.vector.tensor_add(e2[:], e2[:], mv[:, 1:2])
                    nc.vector.tensor_mul(e2[:], e2[:], dq14[:, tb:tb + 1])
                    rms = sc_p.tile([128, 1], F32, tag="rms2")
                    nc.scalar.activation(out=rms[:], in_=e2[:],
                                         func=mybir.ActivationFunctionType.Sqrt,
                                         bias=eps_n[:], scale=1.0)
                    rinv = sc_p.tile([128, 1], F32, tag="rinv2")
                    nc.vector.reciprocal(out=rinv[:], in_=rms[:])
                    dr = sc_p.tile([128, 1], F32, tag="dr")   # dq1^2 * rinv2
                    nc.vector.tensor_mul(dr[:], dq1sq[:, tb:tb + 1], rinv[:])
                    am = sc_p.tile([128, 1], F32, tag="am2s")
                    nc.vector.tensor_reduce(out=am[:], in_=am2[:, tb, :],
                                            axis=mybir.AxisListType.X,
                                            op=mybir.AluOpType.max)
                    nc.vector.tensor_mul(am[:], am[:], dr[:])
                    c2 = sc_p.tile([128, 1], F32, tag="c2")
                    nc.vector.tensor_scalar_max(c2[:], am[:], EPS_SCALE)
                    ic2 = sc_p.tile([128, 1], F32, tag="ic2")
                    nc.vector.reciprocal(out=ic2[:], in_=c2[:])
                    tau = sc_p.tile([128, 1], F32, tag="tau")
                    nc.vector.tensor_mul(tau[:], dr[:], ic2[:])
                    nc.vector.tensor_scalar_mul(tau[:], tau[:], 127.0)
                    dq2 = dq2_p.tile([128, 1], F32, tag="dq2k", name=f"dq2_{tb}")
                    nc.vector.tensor_mul(dq2[:], c2[:], ws_rep[:, 1:2])
                    dq2_tiles.append(dq2)
                    # round(hg * tau) in halves, magic-number RNE, in place on hg
                    for h in range(2):
                        hs = slice(h * (F // 2), (h + 1) * (F // 2))
                        nc.vector.tensor_scalar(hg[:, tb, hs], hg[:, tb, hs],
                                                tau[:], MAGIC,
                                                op0=mybir.AluOpType.mult,
                                                op1=mybir.AluOpType.add)
                        x2b = x2_p.tile([128, F // 2], BF16, tag="x2b")
                        nc.scalar.activation(out=x2b[:], in_=hg[:, tb, hs],
                                             func=mybir.ActivationFunctionType.Copy,
                                             bias=-MAGIC, scale=1.0)
                        if pe_transpose:
                            for j in range(N_OCH // 2):
                                och = h * (N_OCH // 2) + j
                                pt = pst_p.tile([128, 128], BF16, tag="pst",
                                                name=f"pt2_{j}")
                                nc.tensor.transpose(
                                    pt[:], x2b[:, j * 128:(j + 1) * 128], ident[:])
                                nc.vector.tensor_copy(
                                    out=x2T[:, och, tb * TB:(tb + 1) * TB], in_=pt[:])
                        else:
                            nc.sync.dma_start(
                                out=x2_stage[tb * TB:(tb + 1) * TB, hs], in_=x2b[:])
                return (x2T if pe_transpose else x2_stage), dq2_tiles

            def matmul2(g, x2_src, dq2_tiles):
                if pe_transpose:
                    x2T = x2_src
                else:
                    x2T = x2t_p.tile([128, N_OCH, TG], BF16, tag="x2T")
                    for och in range(N_OCH):
                        if fake_transpose:
                            nc.sync.dma_start(out=x2T[:, och, :],
                                              in_=x2_src[0:128, 0:TG])
                        else:
                            nc.sync.dma_start(out=x2T[:, och, :],
                                              in_=x2_src[:, och * 128:(och + 1) * 128],
                                              transpose=True)
                for dsl in range(N_DSL):
                    pys = [ps2_p.tile([128, 512], F32, tag="ps2", name=f"ps2_{tb}")
                           for tb in range(NTB_G)]
                    NQ = N_OCH // 4
                    w2h = [None] * 4
                    for h in range(4):
                        w2h[h] = w2_p.tile([128, NQ, 512], FP8, tag="w2t",
                                           name=f"w2t_{h}")
                        nc.scalar.dma_start(
                            out=w2h[h][:],
                            in_=w2_ext[dsl, :, h * NQ:(h + 1) * NQ, :])
                    for och in range(N_OCH):
                        h, j = divmod(och, NQ)
                        for tb in range(NTB_G):
                            nc.tensor.matmul(pys[tb][:],
                                             lhsT=x2T[:, och, tb * TB:(tb + 1) * TB],
                                             rhs=w2h[h][:, j, :],
                                             start=(och == 0), stop=(och == N_OCH - 1))
                    for tb in range(NTB_G):
                        yt = y_p.tile([128, 512], F32, tag="yt")
                        nc.vector.tensor_scalar_mul(yt[:], pys[tb][:], dq2_tiles[tb][:])
                        t0 = g * TG + tb * TB
                        nc.scalar.dma_start(
                            out=y_ext[t0:t0 + TB, dsl * 512:(dsl + 1) * 512],
                            in_=yt[:])

            def pipeline():
                st2_t = [None] * n_g
                am2_t = [None] * n_g
                hg_t = [None] * n_g
                q1_out = [None] * n_g

                def p1(g):
                    xqT, dq1sq, dq14 = q1_out[g]
                    hg = hg_p.tile([128, NTB_G, F], F32, tag="hg")
                    st2 = st_p.tile([128, NTB_G, N_ORNG, 6], F32, tag="st2")
                    am2 = st_p.tile([128, NTB_G, N_ORNG], F32, tag="am2")
                    matmul1(g, xqT, hg, st2, am2)
                    hg_t[g], st2_t[g], am2_t[g] = hg, st2, am2

                q1_out[0] = quant1(0)
                if n_g > 1:
                    q1_out[1] = quant1(1)
                p1(0)
                for g in range(n_g):
                    xqT, dq1sq, dq14 = q1_out[g]
                    x2s, dq2s = quant2(g, hg_t[g], st2_t[g], am2_t[g], dq1sq, dq14)
                    if g + 2 < n_g:
                        q1_out[g + 2] = quant1(g + 2)
                    if g + 1 < n_g:
                        p1(g + 1)
                    matmul2(g, x2s, dq2s)


            import contextlib
            loop_ctx = tc.For_i(0, replicas, 1) if replicas > 0 else contextlib.nullcontext()
            with loop_ctx:
                pipeline()

    nc.finalize()
    return nc


_NC_CACHE: dict = {}


def _get_nc(t_core: int, unit_g: bool = True):
    key = (t_core, unit_g)
    if key not in _NC_CACHE:
        _NC_CACHE[key] = build_nc(t_core, unit_g)
    return _NC_CACHE[key]


def _prep_weights(w1: np.ndarray, w2: np.ndarray):
    """Host ternarization + tiling. Returns (w1p, w2p, wsc)."""
    def tern(w):
        ws = max(float(np.mean(np.abs(w.astype(np.float64)))), EPS_SCALE)
        t = np.clip(np.round(w.astype(np.float64) / ws), -1, 1).astype(np.float32)
        return t, ws

    t1, ws1 = tern(w1)          # [F, D]
    t2, ws2 = tern(w2)          # [D, F]
    # matmul1 moving tiles: [o_rng][p=i%128][i_chunk][o_in 512] of w1T[i,o]
    w1T = np.ascontiguousarray(t1.T)                    # [D, F]
    w1p = (w1T.reshape(N_I, 128, N_ORNG, ORNG)          # (i_c, p, o_rng, o_in)
              .transpose(2, 1, 0, 3).astype(NP_FP8))    # [o_rng, p, i_c, o_in]
    w1p = np.ascontiguousarray(w1p)
    # matmul2 moving tiles: [d_slice][o_chunk][p=o%128][d_in 512] of w2T[o,d]
    w2p = (t2.reshape(N_DSL, 512, N_OCH, 128)           # (dsl, d_in, o_c, p)
             .transpose(0, 3, 2, 1).astype(NP_FP8))     # [dsl, p, o_c, d_in]
    w2p = np.ascontiguousarray(w2p)
    wsc = np.array([ws1 / 127.0, ws2 / 127.0], dtype=np.float32)
    return w1p, w2p, wsc


def kernel(x: np.ndarray, w1: np.ndarray, g1: np.ndarray,
           w2: np.ndarray, g2: np.ndarray) -> np.ndarray:
    x = np.asarray(x, dtype=np.float32)
    b, s, d = x.shape
    assert (b, s, d) == (B, S, D), (b, s, d)
    w1p, w2p, wsc = _prep_weights(np.asarray(w1, np.float32),
                                  np.asarray(w2, np.float32))
    g1 = np.asarray(g1, np.float32)
    g2f = np.asarray(g2, np.float32)
    unit_g = bool(np.all(g1 == 1.0) and np.all(g2f == 1.0))

    xt = x.reshape(T_TOTAL, D)
    nc = _get_nc(T_CORE, unit_g)
    in_maps = []
    for c in range(N_CORES):
        in_maps.append({
            "x": np.ascontiguousarray(xt[c * T_CORE:(c + 1) * T_CORE]),
            "w1p": w1p, "w2p": w2p, "g1": g1, "g2f": g2f, "wsc": wsc,
        })
    res = run_bass_kernel_spmd(nc, in_maps, list(range(N_CORES)))
    outs = [np.asarray(res.results[c]["y"], np.float32) for c in range(N_CORES)]
    y = np.concatenate(outs, axis=0).reshape(B, S, D)
    return y

